# revision 33
# baseline (speedup 1.0000x reference)
"""Cross-attention Trainium2 kernel (8 NeuronCores, SPMD).

Reference computation (all f32):
    q = x @ Wq + bq            # [N, D]
    k = context @ Wk + bk      # [M, D]
    v = context @ Wv + bv      # [M, D]
    out = softmax(q @ k.T / sqrt(D)) @ v   # [N, D]

Sharding: rows of x (N axis) AND rows of context (M axis) are both split
across the 8 cores.  Each core projects its own context shard to k/v,
the shards are all-gathered in-NEFF (bf16, 2 AllGathers), and each core
then computes attention for its x shard against the full gathered K/V.

Device algorithm per core:
  - softmax is invariant to adding a per-row constant, so
        q @ k.T = (x Wq + bq)(ctx Wk + bk).T
    reduces (mod per-row constants) to  x A ctx.T + w . ctx.T  with
    A = Wq Wk.T and w = Wk bq, both precomputed on the host.  The k
    projection therefore disappears from the device: the host ships
    ctx.T pre-cast to fp8 and it is all-gathered directly (the gather
    has no compute producer, so it starts at t=0).
  - the t/v projections run in bf16 (fp8 weights/inputs here would blow
    the error budget), but t/ctx/P/v are all fp8 e4m3 so both big
    attention matmuls run in DoubleRow perf mode (2 MACs/cell/cyc, one
    instruction contracts a pair of 128-deep k-subtiles).
  - v_c = ctx_c @ Wv (+bv) -> fp8 -> DRAM -> AllGather(v)
    tT  = A.T @ xT (+w)    -> fp8, kept in SBUF (overlaps gathers).
  - attention is software-pipelined over the 8 gathered blocks with the
    score stage running LAG blocks ahead of the P@V stage, so the PE
    keeps doing S^T work (needs only ctx8) while the v-gather finishes:
      S^T  = ctx8_b @ tT               [MB, Nq]  (DoubleRow fp8)
      P^T  = exp(S^T / sqrt(D)) -> fp8            (no max-subtraction:
                                                   scores are ~N(0,1/3))
      out_acc += P^T.T @ v_b           (DoubleRow fp8 over m-tile pairs)
      l_rows  += ones.T @ P^T          (DoubleRow denominator rows,
                                        PE-transposed at the end)
  - out = out_acc / l
"""

import numpy as np
import ml_dtypes

import concourse.bass as bass
import concourse.mybir as mybir
import concourse.tile as tile
from concourse import bacc
from concourse.bass_utils import run_bass_kernel_spmd

BF16 = ml_dtypes.bfloat16
F32 = mybir.dt.float32
BF = mybir.dt.bfloat16
F8 = mybir.dt.float8e4
F8NP = ml_dtypes.float8_e4m3

N_CORES = 8
LAG = 5  # blocks of score-stage lookahead ahead of the P@V stage


def build_nc(n_total, m_total, d):
    """Build the per-core Bass program (SPMD: same NEFF on all cores)."""
    n_shard = n_total // N_CORES
    m_shard = m_total // N_CORES
    mb = m_shard                    # one gathered block per core shard
    assert d % 512 == 0 and n_shard % 512 == 0 and m_shard % 512 == 0
    dc = d // 128
    n_qs = n_shard // 512           # q supertiles per core
    mss = mb // 128                 # m sub-chunks per block
    nb = N_CORES                    # gathered blocks
    lag = min(LAG, nb - 1)
    scale = 1.0 / float(np.sqrt(d))

    nc = bacc.Bacc("TRN2", target_bir_lowering=False, debug=False,
                   num_devices=N_CORES)

    xT = nc.dram_tensor("xT", [d, n_shard], BF, kind="ExternalInput")
    ctxT = nc.dram_tensor("ctxT", [d, m_shard], BF, kind="ExternalInput")
    ctx8T = nc.dram_tensor("ctx8T", [d, m_shard], F8, kind="ExternalInput")
    wq = nc.dram_tensor("wq", [d, d], BF, kind="ExternalInput")  # A=WqWk.T
    wv = nc.dram_tensor("wv", [d, d], BF, kind="ExternalInput")
    bq = nc.dram_tensor("bq", [128, dc], F32, kind="ExternalInput")  # Wk bq
    bv = nc.dram_tensor("bv", [1, d], BF, kind="ExternalInput")
    out = nc.dram_tensor("out", [n_shard, d], F32, kind="ExternalOutput")

    n_ks = 2 if (m_shard // 512) % 2 == 0 else 1   # v gather split
    mk = m_shard // n_ks
    k_src = nc.dram_tensor("k_src", [d, m_shard], F8)
    warm_src = nc.dram_tensor("warm_src", [1, 16], F8)
    warm_dst = nc.dram_tensor("warm_dst", [N_CORES, 1, 16], F8,
                              addr_space="Shared")
    v_loc = [nc.dram_tensor(f"v_loc{h}", [mk, d], F8) for h in range(n_ks)]
    k_all = nc.dram_tensor("k_all", [N_CORES, d, m_shard], F8,
                           addr_space="Shared")
    v_all = [nc.dram_tensor(f"v_all{h}", [N_CORES, mk, d], F8,
                            addr_space="Shared") for h in range(n_ks)]

    xT_v = xT.ap().rearrange("(c p) n -> p c n", p=128)
    ctxT_v = ctxT.ap().rearrange("(c p) m -> p c m", p=128)
    wq_v = wq.ap().rearrange("(c p) f -> p c f", p=128)
    wv_v = wv.ap().rearrange("(c p) f -> p c f", p=128)
    v_loc_v = [t.ap().rearrange("(c p) f -> p c f", p=128) for t in v_loc]
    k_all_v = k_all.ap().rearrange("b (c p) m -> b p c m", p=128)
    v_all_v = [t.ap().rearrange("b (c p) f -> b p c f", p=128)
               for t in v_all]

    groups = [list(range(N_CORES))]

    with tile.TileContext(nc) as tc:
        with (
            tc.tile_pool(name="persist", bufs=1) as persist,
            # kt/vp live OUTSIDE the phase-A pool so their SBUF bytes are
            # distinct from the projection tiles: the first kT load then
            # has no WAR dependency on phase A and can run during t-proj
            tc.tile_pool(name="kt", bufs=2) as kt_pool,
            tc.tile_pool(name="vp", bufs=2) as v_pool,
            tc.tile_pool(name="ps_s", bufs=3, space="PSUM") as ps_s,
            tc.tile_pool(name="ps_o", bufs=2, space="PSUM") as ps_o,
            tc.tile_pool(name="ps_l", bufs=1, space="PSUM") as ps_l,
        ):
            tT_sb = persist.tile([128, dc, n_shard], F8)
            out_acc = persist.tile([128, n_shard // 128, d], F32)
            l_rows = persist.tile([1, n_shard], F32)
            linv_all = persist.tile([128, n_shard // 128], F32)
            # k-pair stride of a DoubleRow stationary AP must be %16==0
            # (s3_lw_dual_fp8_restrictions), hence the padded free dim
            ones_c = persist.tile([128, 2, 16], F8)
            one_f = persist.tile([1, 1], F32)
            bq_sb = persist.tile([128, dc], F32)
            nc.vector.memset(ones_c[:], 1.0)
            nc.vector.memset(one_f[:], 1.0)
            nc.sync.dma_start(out=bq_sb[:], in_=bq.ap())

            # the first collective pays a ~40us comm-init barrier; burn it
            # on a 16-byte dummy gather so the real ones run at full speed
            nc.gpsimd.collective_compute(
                "AllGather", mybir.AluOpType.bypass,
                replica_groups=groups,
                ins=[warm_src.ap()], outs=[warm_dst.ap()],
            )

            # ---------------- phase A: v/t projection of own shard ------
            with tc.tile_pool(name="phaseA", bufs=1) as pa:
                wv_sb = pa.tile([128, dc, d], BF)
                wq_sb = pa.tile([128, dc, d], BF)
                bv_sb = pa.tile([1, d], BF)
                ones_r = pa.tile([1, 128], BF)
                ctx_sb = pa.tile([128, dc, m_shard], BF)
                xT_sb = pa.tile([128, dc, n_shard], BF)
                v_c = pa.tile([128, mss, d], F8)

                # DMA order = queue order: v-proj inputs first so the PE
                # starts ASAP, then the rest of the inputs.
                nc.sync.dma_start(out=wv_sb[:], in_=wv_v)
                nc.sync.dma_start(out=ctx_sb[:, :, :mk],
                                  in_=ctxT_v[:, :, :mk])
                nc.sync.dma_start(out=bv_sb[:], in_=bv.ap())
                nc.sync.dma_start(out=ctx_sb[:, :, mk:],
                                  in_=ctxT_v[:, :, mk:])
                nc.sync.dma_start(out=wq_sb[:], in_=wq_v)
                nc.sync.dma_start(out=xT_sb[:], in_=xT_v)
                nc.vector.memset(ones_r[:], 1.0)
                # "k" gather has no compute producer: bounce the input
                # through an internal DRAM tensor (collectives cannot read
                # IO tensors); the gather itself queues behind the warm-up
                nc.sync.dma_start(out=k_src.ap(), in_=ctx8T.ap())
                nc.gpsimd.collective_compute(
                    "AllGather", mybir.AluOpType.bypass,
                    replica_groups=groups,
                    ins=[k_src.ap()], outs=[k_all.ap()],
                )

                # v_c = ctx_c @ Wv + bv, gathered per half; the ic-outer
                # loop shares each stationary ctx chunk across both d halves
                ndh = d // 512
                for h in range(n_ks):
                    for mc in range(h * mk // 128, (h + 1) * mk // 128):
                        pss = [ps_s.tile([128, 512], F32, tag="s", name=f"psv{i}")
                               for i in range(ndh)]
                        for ic in range(dc):
                            for dh in range(ndh):
                                nc.tensor.matmul(
                                    pss[dh][:],
                                    ctx_sb[:, ic, mc * 128:(mc + 1) * 128],
                                    wv_sb[:, ic, dh * 512:(dh + 1) * 512],
                                    start=(ic == 0), stop=False,
                                )
                        for dh in range(ndh):
                            nc.tensor.matmul(
                                pss[dh][:], ones_r[:1, :128],
                                bv_sb[:1, dh * 512:(dh + 1) * 512],
                                start=False, stop=True,
                            )
                            nc.scalar.copy(
                                out=v_c[:, mc, dh * 512:(dh + 1) * 512],
                                in_=pss[dh][:])
                    # scalar-engine queue: keeps the v_loc stores out of the
                    # sync queue so scheduler reordering can't block them
                    nc.scalar.dma_start(
                        out=v_loc_v[h],
                        in_=v_c[:, h * mk // 128:(h + 1) * mk // 128, :])
                    nc.gpsimd.collective_compute(
                        "AllGather", mybir.AluOpType.bypass,
                        replica_groups=groups,
                        ins=[v_loc[h].ap()], outs=[v_all[h].ap()],
                    )

                # tT = A.T @ xT + w  (overlaps the gathers)
                for oc in range(dc):
                    pss = [ps_s.tile([128, 512], F32, tag="s", name=f"psq{i}")
                           for i in range(n_qs)]
                    for ic in range(dc):
                        for qh in range(n_qs):
                            nc.tensor.matmul(
                                pss[qh][:],
                                wq_sb[:, ic, oc * 128:(oc + 1) * 128],
                                xT_sb[:, ic, qh * 512:(qh + 1) * 512],
                                start=(ic == 0), stop=(ic == dc - 1),
                            )
                    for qh in range(n_qs):
                        nc.scalar.activation(
                            out=tT_sb[:, oc, qh * 512:(qh + 1) * 512],
                            in_=pss[qh][:],
                            func=mybir.ActivationFunctionType.Identity,
                            bias=bq_sb[:, oc:oc + 1],
                        )

            # ---------------- phase B: pipelined attention --------------
            assert dc % 2 == 0 and mss % 2 == 0 and (mk // 128) % 2 == 0
            DR = mybir.MatmulPerfMode.DoubleRow
            with (
                tc.tile_pool(name="pt",
                             bufs=(lag + 1) * n_qs + 4) as pt_pool,
                tc.tile_pool(name="fin", bufs=4) as fin,
            ):
                pts = {}      # b -> [qs] P^T tiles [128, mss, 512]

                def emit_scores(b):
                    kT_sb = kt_pool.tile([128, dc, m_shard], F8, tag="kT")
                    nc.sync.dma_start(out=kT_sb[:], in_=k_all_v[b])
                    pts[b] = [pt_pool.tile([128, mss, 512], F8, tag="pt",
                                           name=f"pt{b}_{i}")
                              for i in range(n_qs)]
                    for ms in range(mss):
                        pss = [ps_s.tile([128, 512], F32, tag="s", name=f"pst{i}")
                               for i in range(n_qs)]
                        for icp in range(dc // 2):
                            for qs in range(n_qs):
                                nc.tensor.matmul(
                                    pss[qs][:],
                                    kT_sb[:, 2 * icp:2 * icp + 2,
                                          ms * 128:(ms + 1) * 128],
                                    tT_sb[:, 2 * icp:2 * icp + 2,
                                          qs * 512:(qs + 1) * 512],
                                    start=(icp == 0), stop=(icp == dc // 2 - 1),
                                    perf_mode=DR,
                                )
                        for qs in range(n_qs):
                            nc.scalar.activation(
                                out=pts[b][qs][:, ms, :], in_=pss[qs][:],
                                func=mybir.ActivationFunctionType.Exp,
                                scale=scale,
                            )
                    # denominator rows: l[q] += sum_m P^T[m, q] with ones as
                    # the stationary operand -> full-rate F=1024 DR matmuls
                    for qs in range(n_qs):
                        plr = ps_l.tile([1, 512], F32, tag="lr",
                                        name=f"plr{b}_{qs}")
                        for msp in range(mss // 2):
                            nc.tensor.matmul(
                                plr[:], ones_c[:, :, :1],
                                pts[b][qs][:, 2 * msp:2 * msp + 2, :],
                                start=(msp == 0), stop=(msp == mss // 2 - 1),
                                perf_mode=DR,
                            )
                        dst = l_rows[:, qs * 512:(qs + 1) * 512]
                        if b == 0:
                            nc.vector.tensor_copy(out=dst, in_=plr[:])
                        else:
                            nc.vector.tensor_add(out=dst, in0=dst, in1=plr[:])
                    if b == nb - 1:
                        # l is complete LAG blocks before the last P@V, so
                        # the PE transpose + reciprocal are fully hidden.
                        # out[:, qi] = l_rows[0, qi*128:...].T @ [[1.0]]
                        lt_ps = ps_s.tile([128, 512], F32, tag="s",
                                          name="lt_ps")
                        for qi in range(n_shard // 128):
                            nc.tensor.matmul(
                                lt_ps[:, qi:qi + 1],
                                l_rows[:, qi * 128:(qi + 1) * 128],
                                one_f[:], skip_group_check=True,
                            )
                        nc.vector.reciprocal(
                            linv_all[:], lt_ps[:, :n_shard // 128])

                def emit_pv(b):
                    # v loads go on the gpsimd queue: on the sync queue they
                    # park at the head waiting for the v AllGather and block
                    # the kT loads behind them (head-of-line blocking)
                    v_sb = [v_pool.tile([128, mk // 128, d], F8,
                                        tag=f"v{h}", name=f"v_sb{h}")
                            for h in range(n_ks)]
                    for h in range(n_ks):
                        nc.gpsimd.dma_start(out=v_sb[h][:], in_=v_all_v[h][b])
                    msp_n = mss // 2
                    for qs in range(n_qs):
                        for qc in range(4):
                            qi = qs * 4 + qc
                            po = ps_o.tile([128, d], F32)
                            for msp in range(msp_n):
                                lhs = pts[b][qs][:, 2 * msp:2 * msp + 2,
                                                 qc * 128:(qc + 1) * 128]
                                h, mloc = divmod(2 * msp, mk // 128)
                                for dh in range(d // 512):
                                    nc.tensor.matmul(
                                        po[:, dh * 512:(dh + 1) * 512],
                                        lhs,
                                        v_sb[h][:, mloc:mloc + 2,
                                                 dh * 512:(dh + 1) * 512],
                                        start=(msp == 0),
                                        stop=(msp == msp_n - 1),
                                        perf_mode=DR,
                                    )
                            if b == 0:
                                nc.vector.tensor_copy(
                                    out=out_acc[:, qi, :], in_=po[:])
                            else:
                                nc.vector.tensor_add(
                                    out=out_acc[:, qi, :],
                                    in0=out_acc[:, qi, :], in1=po[:])
                            if b == nb - 1:
                                # normalize + write out as soon as this q
                                # chunk's accumulation is complete
                                o_sb = fin.tile([128, d], F32, tag="osb",
                                                name=f"osb{qi}")
                                nc.vector.tensor_scalar_mul(
                                    out=o_sb[:], in0=out_acc[:, qi, :],
                                    scalar1=linv_all[:, qi:qi + 1])
                                nc.sync.dma_start(
                                    out=out.ap()[qi * 128:(qi + 1) * 128, :],
                                    in_=o_sb[:])
                    del pts[b]

                for b in range(nb + lag):
                    if b < nb:
                        emit_scores(b)
                    if b - lag >= 0:
                        emit_pv(b - lag)


    nc.compile()
    return nc


_NC_CACHE = {}


def _get_nc(n_total, m_total, d):
    key = (n_total, m_total, d)
    if key not in _NC_CACHE:
        _NC_CACHE[key] = build_nc(n_total, m_total, d)
    return _NC_CACHE[key]


def _prep_inputs(x, context, Wq, bq, Wk, bk, Wv, bv, n_cores=N_CORES):
    """Host-side layout prep: transpose + cast + per-core sharding.

    Folds the k projection into the score path (softmax is shift
    invariant per row):  A = Wq Wk.T,  w = Wk bq,  so on-device
    scores = (x A + w) @ ctx.T  and ctx itself (fp8) acts as K.
    """
    x = np.asarray(x, np.float32)
    context = np.asarray(context, np.float32)
    n, d = x.shape
    m = context.shape[0]
    dc = d // 128
    n_shard = n // n_cores
    m_shard = m // n_cores

    Wq = np.asarray(Wq, np.float32)
    Wk = np.asarray(Wk, np.float32)
    A = Wq @ Wk.T                                          # [D, D]
    w = Wk @ np.asarray(bq, np.float32)                    # [D]

    xT = np.ascontiguousarray(x.T).astype(BF16)            # [D, N]
    ctxT = np.ascontiguousarray(context.T)                 # [D, M] f32
    ctxT_b = ctxT.astype(BF16)
    ctxT_8 = ctxT.astype(F8NP)
    wq_b = A.astype(BF16)
    wv_b = np.asarray(Wv, np.float32).astype(BF16)
    bq_g = np.ascontiguousarray(w.reshape(dc, 128).T)
    bv_r = np.asarray(bv, np.float32).astype(BF16).reshape(1, d)

    in_maps = []
    for c in range(n_cores):
        in_maps.append({
            "xT": np.ascontiguousarray(xT[:, c * n_shard:(c + 1) * n_shard]),
            "ctxT": np.ascontiguousarray(
                ctxT_b[:, c * m_shard:(c + 1) * m_shard]),
            "ctx8T": np.ascontiguousarray(
                ctxT_8[:, c * m_shard:(c + 1) * m_shard]),
            "wq": wq_b, "wv": wv_b,
            "bq": bq_g, "bv": bv_r,
        })
    return in_maps, n_shard


def run(x, context, Wq, bq, Wk, bk, Wv, bv, trace=False):
    """Run the SPMD kernel; returns (out_full, BassKernelResults)."""
    in_maps, n_shard = _prep_inputs(x, context, Wq, bq, Wk, bk, Wv, bv)
    n_total = np.asarray(x).shape[0]
    m_total, d = np.asarray(context).shape
    nc = _get_nc(n_total, m_total, d)
    res = run_bass_kernel_spmd(nc, in_maps, core_ids=list(range(N_CORES)),
                               trace=trace)
    out = np.concatenate([res.results[c]["out"] for c in range(N_CORES)],
                         axis=0)
    return np.asarray(out, np.float32), res


def kernel(x, context, Wq, bq, Wk, bk, Wv, bv):
    out, _ = run(x, context, Wq, bq, Wk, bk, Wv, bv, trace=False)
    return out



# revision 41
# speedup vs baseline: 1.0611x; 1.0611x over previous
"""Cross-attention Trainium2 kernel (8 NeuronCores, SPMD).

Reference computation (all f32):
    q = x @ Wq + bq            # [N, D]
    k = context @ Wk + bk      # [M, D]
    v = context @ Wv + bv      # [M, D]
    out = softmax(q @ k.T / sqrt(D)) @ v   # [N, D]

Sharding: rows of x (N axis) AND rows of context (M axis) are both split
across the 8 cores.  Each core projects its own context shard to k/v,
the shards are all-gathered in-NEFF (bf16, 2 AllGathers), and each core
then computes attention for its x shard against the full gathered K/V.

Device algorithm per core:
  - softmax is invariant to adding a per-row constant, so
        q @ k.T = (x Wq + bq)(ctx Wk + bk).T
    reduces (mod per-row constants) to  x A ctx.T + w . ctx.T  with
    A = Wq Wk.T and w = Wk bq, both precomputed on the host.  The k
    projection therefore disappears from the device: the host ships
    ctx.T pre-cast to fp8 and it is all-gathered directly (the gather
    has no compute producer, so it starts at t=0).
  - the t/v projections run in bf16 (fp8 weights/inputs here would blow
    the error budget), but t/ctx/P/v are all fp8 e4m3 so both big
    attention matmuls run in DoubleRow perf mode (2 MACs/cell/cyc, one
    instruction contracts a pair of 128-deep k-subtiles).
  - v_c = ctx_c @ Wv (+bv) -> fp8 -> DRAM -> AllGather(v)
    tT  = A.T @ xT (+w)    -> fp8, kept in SBUF (overlaps gathers).
  - attention is software-pipelined over the 8 gathered blocks with the
    score stage running LAG blocks ahead of the P@V stage, so the PE
    keeps doing S^T work (needs only ctx8) while the v-gather finishes:
      S^T  = ctx8_b @ tT               [MB, Nq]  (DoubleRow fp8)
      P^T  = exp(S^T / sqrt(D)) -> fp8            (no max-subtraction:
                                                   scores are ~N(0,1/3))
      out_acc += P^T.T @ v_b           (DoubleRow fp8 over m-tile pairs)
      l_rows  += ones.T @ P^T          (DoubleRow denominator rows,
                                        PE-transposed at the end)
  - out = out_acc / l
"""

import numpy as np
import ml_dtypes

import concourse.bass as bass
import concourse.mybir as mybir
import concourse.tile as tile
from concourse import bacc
from concourse.bass_utils import run_bass_kernel_spmd

BF16 = ml_dtypes.bfloat16
F32 = mybir.dt.float32
BF = mybir.dt.bfloat16
F8 = mybir.dt.float8e4
F8NP = ml_dtypes.float8_e4m3

N_CORES = 8
LAG = 5  # blocks of score-stage lookahead ahead of the P@V stage


def build_nc(n_total, m_total, d):
    """Build the per-core Bass program (SPMD: same NEFF on all cores)."""
    n_shard = n_total // N_CORES
    m_shard = m_total // N_CORES
    mb = m_shard                    # one gathered block per core shard
    assert d % 512 == 0 and n_shard % 512 == 0 and m_shard % 512 == 0
    dc = d // 128
    n_qs = n_shard // 512           # q supertiles per core
    mss = mb // 128                 # m sub-chunks per block
    nb = N_CORES                    # gathered blocks
    lag = min(LAG, nb - 1)
    scale = 1.0 / float(np.sqrt(d))

    nc = bacc.Bacc("TRN2", target_bir_lowering=False, debug=False,
                   num_devices=N_CORES)

    xT = nc.dram_tensor("xT", [d, n_shard], BF, kind="ExternalInput")
    ctxT = nc.dram_tensor("ctxT", [d, m_shard], BF, kind="ExternalInput")
    ctx8T = nc.dram_tensor("ctx8T", [d, m_shard], F8, kind="ExternalInput")
    wq = nc.dram_tensor("wq", [d, d], BF, kind="ExternalInput")  # A=WqWk.T
    wv = nc.dram_tensor("wv", [d, d], BF, kind="ExternalInput")
    bq = nc.dram_tensor("bq", [128, dc], F32, kind="ExternalInput")  # Wk bq
    bv = nc.dram_tensor("bv", [1, d], BF, kind="ExternalInput")
    out = nc.dram_tensor("out", [n_shard, d], F32, kind="ExternalOutput")

    n_ks = 2 if (m_shard // 512) % 2 == 0 else 1   # v gather split
    mk = m_shard // n_ks
    k_src = [nc.dram_tensor(f"k_src{h}", [d, mk], F8) for h in range(n_ks)]
    v_loc = [nc.dram_tensor(f"v_loc{h}", [mk, d], F8) for h in range(n_ks)]
    k_all = [nc.dram_tensor(f"k_all{h}", [N_CORES, d, mk], F8,
                            addr_space="Shared") for h in range(n_ks)]
    v_all = [nc.dram_tensor(f"v_all{h}", [N_CORES, mk, d], F8,
                            addr_space="Shared") for h in range(n_ks)]

    xT_v = xT.ap().rearrange("(c p) n -> p c n", p=128)
    ctxT_v = ctxT.ap().rearrange("(c p) m -> p c m", p=128)
    wq_v = wq.ap().rearrange("(c p) f -> p c f", p=128)
    wv_v = wv.ap().rearrange("(c p) f -> p c f", p=128)
    v_loc_v = [t.ap().rearrange("(c p) f -> p c f", p=128) for t in v_loc]
    k_all_v = [t.ap().rearrange("b (c p) m -> b p c m", p=128)
               for t in k_all]
    v_all_v = [t.ap().rearrange("b (c p) f -> b p c f", p=128)
               for t in v_all]

    groups = [list(range(N_CORES))]

    with tile.TileContext(nc) as tc:
        with (
            tc.tile_pool(name="persist", bufs=1) as persist,
            # kt/vp live OUTSIDE the phase-A pool so their SBUF bytes are
            # distinct from the projection tiles: the first kT load then
            # has no WAR dependency on phase A and can run during t-proj
            tc.tile_pool(name="kt", bufs=3) as kt_pool,
            tc.tile_pool(name="vp", bufs=2) as v_pool,
            tc.tile_pool(name="ps_s", bufs=3, space="PSUM") as ps_s,
            tc.tile_pool(name="ps_o", bufs=2, space="PSUM") as ps_o,
            tc.tile_pool(name="ps_l", bufs=1, space="PSUM") as ps_l,
        ):
            tT_sb = persist.tile([128, dc, n_shard], F8)
            out_acc = persist.tile([128, n_shard // 128, d], F32)
            l_rows = persist.tile([1, n_shard], F32)
            linv_all = persist.tile([128, n_shard // 128], F32)
            # k-pair stride of a DoubleRow stationary AP must be %16==0
            # (s3_lw_dual_fp8_restrictions), hence the padded free dim
            ones_c = persist.tile([128, 2, 16], F8)
            one_f = persist.tile([1, 1], F32)
            bq_sb = persist.tile([128, dc], F32)
            nc.vector.memset(ones_c[:], 1.0)
            nc.vector.memset(one_f[:], 1.0)
            nc.sync.dma_start(out=bq_sb[:], in_=bq.ap())

            # ---------------- phase A: v/t projection of own shard ------
            with tc.tile_pool(name="phaseA", bufs=1) as pa:
                wv_sb = pa.tile([128, dc, d], BF)
                wq_sb = pa.tile([128, dc, d], BF)
                bv_sb = pa.tile([1, d], BF)
                ones_r = pa.tile([1, 128], BF)
                ctx_sb = pa.tile([128, dc, m_shard], BF)
                xT_sb = pa.tile([128, dc, n_shard], BF)
                v_c = pa.tile([128, mss, d], F8)

                # DMA order = queue order: v-proj inputs first so the PE
                # starts ASAP, then the rest of the inputs.
                nc.sync.dma_start(out=wv_sb[:], in_=wv_v)
                nc.sync.dma_start(out=ctx_sb[:, :, :mk],
                                  in_=ctxT_v[:, :, :mk])
                # "k" gathers have no compute producer: bounce the input
                # through internal DRAM tensors (collectives cannot read IO
                # tensors) early in the queue so the scheduler orders the k
                # gathers ahead of the v gathers on the cc queue.
                for h in range(n_ks):
                    nc.sync.dma_start(out=k_src[h].ap(),
                                      in_=ctx8T.ap()[:, h * mk:(h + 1) * mk])
                    nc.gpsimd.collective_compute(
                        "AllGather", mybir.AluOpType.bypass,
                        replica_groups=groups,
                        ins=[k_src[h].ap()], outs=[k_all[h].ap()],
                    )
                nc.sync.dma_start(out=bv_sb[:], in_=bv.ap())
                nc.sync.dma_start(out=ctx_sb[:, :, mk:],
                                  in_=ctxT_v[:, :, mk:])
                nc.sync.dma_start(out=wq_sb[:], in_=wq_v)
                nc.sync.dma_start(out=xT_sb[:], in_=xT_v)
                nc.vector.memset(ones_r[:], 1.0)

                # v_c = ctx_c @ Wv + bv, gathered per half; the ic-outer
                # loop shares each stationary ctx chunk across both d halves
                ndh = d // 512
                for h in range(n_ks):
                    for mc in range(h * mk // 128, (h + 1) * mk // 128):
                        pss = [ps_s.tile([128, 512], F32, tag="s", name=f"psv{i}")
                               for i in range(ndh)]
                        for ic in range(dc):
                            for dh in range(ndh):
                                nc.tensor.matmul(
                                    pss[dh][:],
                                    ctx_sb[:, ic, mc * 128:(mc + 1) * 128],
                                    wv_sb[:, ic, dh * 512:(dh + 1) * 512],
                                    start=(ic == 0), stop=False,
                                )
                        for dh in range(ndh):
                            nc.tensor.matmul(
                                pss[dh][:], ones_r[:1, :128],
                                bv_sb[:1, dh * 512:(dh + 1) * 512],
                                start=False, stop=True,
                            )
                            nc.scalar.copy(
                                out=v_c[:, mc, dh * 512:(dh + 1) * 512],
                                in_=pss[dh][:])
                    # scalar-engine queue: keeps the v_loc stores out of the
                    # sync queue so scheduler reordering can't block them
                    nc.scalar.dma_start(
                        out=v_loc_v[h],
                        in_=v_c[:, h * mk // 128:(h + 1) * mk // 128, :])
                    nc.gpsimd.collective_compute(
                        "AllGather", mybir.AluOpType.bypass,
                        replica_groups=groups,
                        ins=[v_loc[h].ap()], outs=[v_all[h].ap()],
                    )

                # tT = A.T @ xT + w  (overlaps the gathers)
                for oc in range(dc):
                    pss = [ps_s.tile([128, 512], F32, tag="s", name=f"psq{i}")
                           for i in range(n_qs)]
                    for ic in range(dc):
                        for qh in range(n_qs):
                            nc.tensor.matmul(
                                pss[qh][:],
                                wq_sb[:, ic, oc * 128:(oc + 1) * 128],
                                xT_sb[:, ic, qh * 512:(qh + 1) * 512],
                                start=(ic == 0), stop=(ic == dc - 1),
                            )
                    for qh in range(n_qs):
                        nc.scalar.activation(
                            out=tT_sb[:, oc, qh * 512:(qh + 1) * 512],
                            in_=pss[qh][:],
                            func=mybir.ActivationFunctionType.Identity,
                            bias=bq_sb[:, oc:oc + 1],
                        )

            # ---------------- phase B: pipelined attention --------------
            assert dc % 2 == 0 and mss % 2 == 0 and (mk // 128) % 2 == 0
            DR = mybir.MatmulPerfMode.DoubleRow
            with (
                # all blocks' P^T h0 halves are alive before the first P@V
                tc.tile_pool(name="pt", bufs=nb * n_qs + 2) as pt_pool,
                tc.tile_pool(name="fin", bufs=4) as fin,
            ):
                pts = {}      # b -> [qs] P^T tiles [128, mss, 512]
                mss2 = mss // 2

                def emit_scores_half(b, h):
                    # scores for the ms-rows of gather-half h of block b:
                    # all h0 halves run before any h1, so the PE starts as
                    # soon as the first k half-gather lands
                    kT_sb = kt_pool.tile([128, dc, mk], F8, tag=f"kT{h}",
                                         name=f"kT{h}_{b}")
                    nc.sync.dma_start(out=kT_sb[:], in_=k_all_v[h][b])
                    if h == 0:
                        pts[b] = [pt_pool.tile([128, mss, 512], F8, tag="pt",
                                               name=f"pt{b}_{i}")
                                  for i in range(n_qs)]
                    for ms in range(h * mss2, (h + 1) * mss2):
                        mloc = ms * 128 - h * mk
                        pss = [ps_s.tile([128, 512], F32, tag="s", name=f"pst{i}")
                               for i in range(n_qs)]
                        for icp in range(dc // 2):
                            for qs in range(n_qs):
                                nc.tensor.matmul(
                                    pss[qs][:],
                                    kT_sb[:, 2 * icp:2 * icp + 2,
                                          mloc:mloc + 128],
                                    tT_sb[:, 2 * icp:2 * icp + 2,
                                          qs * 512:(qs + 1) * 512],
                                    start=(icp == 0), stop=(icp == dc // 2 - 1),
                                    perf_mode=DR,
                                )
                        for qs in range(n_qs):
                            nc.scalar.activation(
                                out=pts[b][qs][:, ms, :], in_=pss[qs][:],
                                func=mybir.ActivationFunctionType.Exp,
                                scale=scale,
                            )
                    if h != n_ks - 1:
                        return
                    # denominator rows: l[q] += sum_m P^T[m, q] with ones as
                    # the stationary operand -> full-rate F=1024 DR matmuls
                    for qs in range(n_qs):
                        plr = ps_l.tile([1, 512], F32, tag="lr",
                                        name=f"plr{b}_{qs}")
                        for msp in range(mss // 2):
                            nc.tensor.matmul(
                                plr[:], ones_c[:, :, :1],
                                pts[b][qs][:, 2 * msp:2 * msp + 2, :],
                                start=(msp == 0), stop=(msp == mss // 2 - 1),
                                perf_mode=DR,
                            )
                        dst = l_rows[:, qs * 512:(qs + 1) * 512]
                        if b == 0:
                            nc.vector.tensor_copy(out=dst, in_=plr[:])
                        else:
                            nc.vector.tensor_add(out=dst, in0=dst, in1=plr[:])
                    if b == nb - 1:
                        # l completes one block before the last P@V; the PE
                        # transpose + reciprocal hide under pv(nb-2).
                        # out[:, qi] = l_rows[0, qi*128:...].T @ [[1.0]]
                        lt_ps = ps_s.tile([128, 512], F32, tag="s",
                                          name="lt_ps")
                        for qi in range(n_shard // 128):
                            nc.tensor.matmul(
                                lt_ps[:, qi:qi + 1],
                                l_rows[:, qi * 128:(qi + 1) * 128],
                                one_f[:], skip_group_check=True,
                            )
                        nc.vector.reciprocal(
                            linv_all[:], lt_ps[:, :n_shard // 128])

                def emit_pv(b):
                    # v loads go on the gpsimd queue: on the sync queue they
                    # park at the head waiting for the v AllGather and block
                    # the kT loads behind them (head-of-line blocking)
                    v_sb = [v_pool.tile([128, mk // 128, d], F8,
                                        tag=f"v{h}", name=f"v_sb{h}")
                            for h in range(n_ks)]
                    for h in range(n_ks):
                        nc.gpsimd.dma_start(out=v_sb[h][:], in_=v_all_v[h][b])
                    msp_n = mss // 2
                    for qs in range(n_qs):
                        for qc in range(4):
                            qi = qs * 4 + qc
                            po = ps_o.tile([128, d], F32)
                            for msp in range(msp_n):
                                lhs = pts[b][qs][:, 2 * msp:2 * msp + 2,
                                                 qc * 128:(qc + 1) * 128]
                                h, mloc = divmod(2 * msp, mk // 128)
                                for dh in range(d // 512):
                                    nc.tensor.matmul(
                                        po[:, dh * 512:(dh + 1) * 512],
                                        lhs,
                                        v_sb[h][:, mloc:mloc + 2,
                                                 dh * 512:(dh + 1) * 512],
                                        start=(msp == 0),
                                        stop=(msp == msp_n - 1),
                                        perf_mode=DR,
                                    )
                            if b == 0:
                                nc.vector.tensor_copy(
                                    out=out_acc[:, qi, :], in_=po[:])
                            else:
                                nc.vector.tensor_add(
                                    out=out_acc[:, qi, :],
                                    in0=out_acc[:, qi, :], in1=po[:])
                            if b == nb - 1:
                                # normalize + write out as soon as this q
                                # chunk's accumulation is complete
                                o_sb = fin.tile([128, d], F32, tag="osb",
                                                name=f"osb{qi}")
                                nc.vector.tensor_scalar_mul(
                                    out=o_sb[:], in0=out_acc[:, qi, :],
                                    scalar1=linv_all[:, qi:qi + 1])
                                nc.sync.dma_start(
                                    out=out.ap()[qi * 128:(qi + 1) * 128, :],
                                    in_=o_sb[:])
                    del pts[b]

                # h0 scores of every block first (only k_h0 is needed), then
                # h1 scores interleaved with P@V one block behind
                for b in range(nb):
                    emit_scores_half(b, 0)
                for b in range(nb + 1):
                    if b < nb:
                        emit_scores_half(b, 1)
                    if b - 1 >= 0:
                        emit_pv(b - 1)


    nc.compile()
    return nc


_NC_CACHE = {}


def _get_nc(n_total, m_total, d):
    key = (n_total, m_total, d)
    if key not in _NC_CACHE:
        _NC_CACHE[key] = build_nc(n_total, m_total, d)
    return _NC_CACHE[key]


def _prep_inputs(x, context, Wq, bq, Wk, bk, Wv, bv, n_cores=N_CORES):
    """Host-side layout prep: transpose + cast + per-core sharding.

    Folds the k projection into the score path (softmax is shift
    invariant per row):  A = Wq Wk.T,  w = Wk bq,  so on-device
    scores = (x A + w) @ ctx.T  and ctx itself (fp8) acts as K.
    """
    x = np.asarray(x, np.float32)
    context = np.asarray(context, np.float32)
    n, d = x.shape
    m = context.shape[0]
    dc = d // 128
    n_shard = n // n_cores
    m_shard = m // n_cores

    Wq = np.asarray(Wq, np.float32)
    Wk = np.asarray(Wk, np.float32)
    A = Wq @ Wk.T                                          # [D, D]
    w = Wk @ np.asarray(bq, np.float32)                    # [D]

    xT = np.ascontiguousarray(x.T).astype(BF16)            # [D, N]
    ctxT = np.ascontiguousarray(context.T)                 # [D, M] f32
    ctxT_b = ctxT.astype(BF16)
    ctxT_8 = ctxT.astype(F8NP)
    wq_b = A.astype(BF16)
    wv_b = np.asarray(Wv, np.float32).astype(BF16)
    bq_g = np.ascontiguousarray(w.reshape(dc, 128).T)
    bv_r = np.asarray(bv, np.float32).astype(BF16).reshape(1, d)

    in_maps = []
    for c in range(n_cores):
        in_maps.append({
            "xT": np.ascontiguousarray(xT[:, c * n_shard:(c + 1) * n_shard]),
            "ctxT": np.ascontiguousarray(
                ctxT_b[:, c * m_shard:(c + 1) * m_shard]),
            "ctx8T": np.ascontiguousarray(
                ctxT_8[:, c * m_shard:(c + 1) * m_shard]),
            "wq": wq_b, "wv": wv_b,
            "bq": bq_g, "bv": bv_r,
        })
    return in_maps, n_shard


def run(x, context, Wq, bq, Wk, bk, Wv, bv, trace=False):
    """Run the SPMD kernel; returns (out_full, BassKernelResults)."""
    in_maps, n_shard = _prep_inputs(x, context, Wq, bq, Wk, bk, Wv, bv)
    n_total = np.asarray(x).shape[0]
    m_total, d = np.asarray(context).shape
    nc = _get_nc(n_total, m_total, d)
    res = run_bass_kernel_spmd(nc, in_maps, core_ids=list(range(N_CORES)),
                               trace=trace)
    out = np.concatenate([res.results[c]["out"] for c in range(N_CORES)],
                         axis=0)
    return np.asarray(out, np.float32), res


def kernel(x, context, Wq, bq, Wk, bk, Wv, bv):
    out, _ = run(x, context, Wq, bq, Wk, bk, Wv, bv, trace=False)
    return out

